# revision 31
# baseline (speedup 1.0000x reference)
"""Trainium2 Bass kernel for a 12-head dense attention block (BEiT-style
windowed attention with relative-position bias), batch-parallel over 8
NeuronCores.

Shapes (hardcoded): x [64, 197, 768], qkv_w [2304, 768], proj_w [768, 768],
proj_b [768], rel_table [732, 12], rel_index [197, 197] int32.

Sharding: data-parallel over batch — each of the 8 cores handles 8 batch
elements end-to-end; no collectives. Host pre-transposes x and the weights
so the device kernel needs no on-chip transposes:

  phase 1: qkT[2C, M] = wqkvT.T-style matmul producing q,k TRANSPOSED
           ([feature, token]) + v in natural layout ([token, feature]),
           bf16 matmuls (fast weight load).
  phase 2: per (batch, head): scoresT[nk, nq] = kT.T @ qT, exp on the
           scalar engine, relative-position bias applied multiplicatively
           (exp(bias) precomputed on host; kt0 multiply on DVE, kt1 on
           GpSimd to balance engines), softmax denominators via a
           ones-row matmul, reciprocal_approx_fast off psum, attention
           output accumulated TRANSPOSED (outT[d, nq]) and normalized by
           merged psum->SBUF DVE multiplies (2 per head-group, exploiting
           the SIG4 slot adjacency).
  phase 3: yT[c, m] = wprojT-chunk.T @ attn_outT (bf16), proj bias fused
           into the scalar-engine psum evacuation (per-partition bias AP),
           DMA out in [C, M] layout; host transposes back.
"""

import sys
import time

if "/opt/trn_rl_repo" not in sys.path:
    sys.path.insert(0, "/opt/trn_rl_repo")

import numpy as np
import ml_dtypes

import concourse.bass as bass  # noqa: F401  (registers rust bindings)
import concourse.tile as tile
from concourse import bacc, mybir
from concourse.bass_utils import run_bass_kernel_spmd

N_CORES = 8
B, N, C, H, D = 64, 197, 768, 12, 64
BL = B // N_CORES            # 8 batch elements per core
M = BL * N                   # 1576 tokens per core
SCALE = D ** -0.5
NK0 = 128
NK1 = N - NK0                # 69
KC = C // 128                # 6 contraction chunks
MT = 4                       # m-tiles in phase 1 (qk part)
MTS = M // MT                # 394
NT_QK = (2 * C) // 128       # 12 output-feature tiles for q,k
TC3 = 4                      # token chunks in phase 3 (394 each)
TCS = M // TC3               # 394

F32 = mybir.dt.float32
BF16 = mybir.dt.bfloat16

# self-inverse head<->slot permutation within each 4-head group: consecutive
# score matmuls alternate array row-strips (head parity) and run concurrently,
# so they must target different PSUM banks -> interleave slots (0,2,1,3)
SIG4 = (0, 2, 1, 3)


def sig(h):
    return (h // 4) * 4 + SIG4[h % 4]


_COMPILED = {}


def _build_nc():
    nc = bacc.Bacc(
        "TRN2", target_bir_lowering=False, debug=False, num_devices=N_CORES
    )
    xT = nc.declare_dram_parameter("xT", [C, M], BF16, isOutput=False)
    wqkvT = nc.declare_dram_parameter("wqkvT", [C, 3 * C], BF16, isOutput=False)
    wprojT = nc.declare_dram_parameter("wprojT", [C, C], BF16, isOutput=False)
    projb = nc.declare_dram_parameter("projb", [1, C], F32, isOutput=False)
    biasT = nc.declare_dram_parameter("biasT", [2, 128, H, N], BF16, isOutput=False)
    out_d = nc.declare_dram_parameter("out", [C, M], F32, isOutput=True)

    with tile.TileContext(nc) as tc:
        _body(nc, tc, xT, wqkvT, wprojT, projb, biasT, out_d)
    nc.compile()
    return nc


def _body(nc, tc, xT, wqkvT, wprojT, projb, biasT, out_d):
    exp = mybir.ActivationFunctionType.Exp
    ident = mybir.ActivationFunctionType.Identity

    consts = tc.alloc_tile_pool(name="consts", bufs=1)
    ones128 = consts.tile([128, 128], BF16)
    nc.vector.memset(ones128, 1.0)
    # proj bias in column layout: element (p, k) = proj_b[k*128 + p]
    projb_cols = consts.tile([128, KC], F32)
    bias_sb = [consts.tile([128, H, N], BF16, tag=f"bias{t}", name=f"bias{t}") for t in range(2)]
    deferred_dmas = []

    # ---- outputs of phase 1 (persist into phase 2) ----
    qk_pool = tc.alloc_tile_pool(name="qk", bufs=1)
    qkT = [qk_pool.tile([128, M], BF16, tag=f"qk{t}", name=f"qk{t}") for t in range(NT_QK)]
    v_pool = tc.alloc_tile_pool(name="v", bufs=1)
    v_sb = [
        [v_pool.tile([128, C], BF16, tag=f"v{b}_{pt}", name=f"v{b}_{pt}") for pt in range(2)]
        for b in range(BL)
    ]

    ps_mm = tc.alloc_tile_pool(name="psmm", bufs=2, space="PSUM")

    # ---- PE warm-up while the input DMAs land ----
    warm_pool = tc.alloc_tile_pool(name="warm", bufs=1, space="PSUM")
    wtile = warm_pool.tile([128, 512], F32, tag="warm")
    for _ in range(12):
        nc.tensor.matmul(wtile[:, 0:128], ones128[:, :], ones128[:, :],
                         start=True, stop=True)
    warm_pool.release()

    # ---- input DMAs ----
    xt_pool = tc.alloc_tile_pool(name="xt", bufs=1)
    wq_pool = tc.alloc_tile_pool(name="wq", bufs=1)
    xt = [xt_pool.tile([128, M], BF16, tag=f"xt{k}", name=f"xt{k}") for k in range(KC)]
    wq = [wq_pool.tile([128, 3 * C], BF16, tag=f"wq{k}", name=f"wq{k}") for k in range(KC)]
    # two parallel DMA queues: xt chunks on the SP queue, wq chunks on the
    # Activation queue (idle until the first exp).  wq's q,k columns land
    # before its v columns — mt0's v matmuls run ~12us after its qk ones.
    for k in range(KC):
        nc.sync.dma_start(out=xt[k][:, :], in_=xT[k * 128 : (k + 1) * 128, :])
        nc.scalar.dma_start(
            out=wq[k][:, 0 : 2 * C], in_=wqkvT[k * 128 : (k + 1) * 128, 0 : 2 * C]
        )
    for k in range(KC):
        last_in_dma = nc.scalar.dma_start(
            out=wq[k][:, 2 * C : 3 * C],
            in_=wqkvT[k * 128 : (k + 1) * 128, 2 * C : 3 * C],
        )
    # secondary inputs (bias table, proj bias, proj weights) wait for the
    # phase-1 inputs so the startup DMA ramp is as short as possible
    _pb = projb[:, :]
    deferred_dmas.append(
        nc.sync.dma_start(
            out=projb_cols[:, :],
            in_=bass.AP(
                tensor=_pb.tensor, offset=_pb.offset, ap=[[1, 128], [128, KC]]
            ),
        )
    )
    for t in range(2):
        deferred_dmas.append(
            nc.sync.dma_start(out=bias_sb[t][:, :, :], in_=biasT[t, :, :, :])
        )
    wp_pool = tc.alloc_tile_pool(name="wp", bufs=1)
    wp = [wp_pool.tile([128, C], BF16, tag=f"wp{k}", name=f"wp{k}") for k in range(KC)]
    for k in range(KC):
        deferred_dmas.append(
            nc.sync.dma_start(out=wp[k][:, :], in_=wprojT[k * 128 : (k + 1) * 128, :])
        )
    for d in deferred_dmas:
        tile.add_dep_helper(d.ins, last_in_dma.ins, sync=True, reason="defer-input")

    def emit_p1_mt(mt):
        # one phase-1 block: qkT/v for token range [mt*MTS, (mt+1)*MTS)
        # (= batch elements 2mt, 2mt+1); psum copies on DVE (Scalar is
        # reserved for the attention exp + output evacuations)
        ms = slice(mt * MTS, (mt + 1) * MTS)
        for nt in range(NT_QK):
            # mt0 runs while the input chunks are still landing: borrow the
            # (still idle) attention psum pools so up to 6 accumulation
            # groups are in flight and each arriving chunk unlocks 3x more
            # PE work.  Later mts are not DMA-paced; 2 bufs suffice.
            sel = nt % 3 if mt == 0 else 0
            if sel == 1:
                t = ps_sc.tile([128, 4, 256], F32, tag="pssc", name=f"p1s_{nt}")
                ps = t.rearrange("p a b -> p (a b)")[:, 0:MTS]
            elif sel == 2:
                t = ps_po.tile([128, 2, 256], F32, tag="pspo", name=f"p1p_{nt}")
                ps = t.rearrange("p a b -> p (a b)")[:, 0:MTS]
            else:
                ps = ps_mm.tile([128, MTS], F32, tag="ps1", name=f"p1_{mt}_{nt}")[:, :]
            for k in range(KC):
                nc.tensor.matmul(
                    ps,
                    wq[k][:, nt * 128 : (nt + 1) * 128],
                    xt[k][:, ms],
                    start=(k == 0),
                    stop=(k == KC - 1),
                )
            nc.vector.tensor_copy(qkT[nt][:, ms], ps)
        for b in (2 * mt, 2 * mt + 1):
            for pt in range(2):
                psz = NK0 if pt == 0 else NK1
                mofs = b * N + pt * 128
                for nt2 in range(2):
                    ps = ps_mm.tile([128, 384], F32, tag="ps1", name=f"pv_{b}_{pt}_{nt2}")
                    for k in range(KC):
                        nc.tensor.matmul(
                            ps[:psz, :],
                            xt[k][:, mofs : mofs + psz],
                            wq[k][
                                :, 2 * C + nt2 * 384 : 2 * C + (nt2 + 1) * 384
                            ],
                            start=(k == 0),
                            stop=(k == KC - 1),
                        )
                    nc.vector.tensor_copy(
                        v_sb[b][pt][:psz, nt2 * 384 : (nt2 + 1) * 384],
                        ps[:psz, :],
                    )

    # attention output, transposed: aoT[:, c, m] = feature chunk c row block
    ao_pool = tc.alloc_tile_pool(name="ao", bufs=1)
    aoT = ao_pool.tile([128, KC, M], BF16, tag="ao", name="ao")

    # ---- phase 2: attention per batch element ----
    et_pool = tc.alloc_tile_pool(name="et", bufs=2)
    raw_pool = tc.alloc_tile_pool(name="raw", bufs=3)
    ar_pool = tc.alloc_tile_pool(name="ar", bufs=2)
    ps_sc = tc.alloc_tile_pool(name="pssc", bufs=2, space="PSUM")
    ps_po = tc.alloc_tile_pool(name="pspo", bufs=2, space="PSUM")

    ostg_pool = tc.alloc_tile_pool(name="ostg", bufs=5)

    # deferred phase-3 units: each emits one output feature chunk (6 MMs +
    # fused-bias psum evacuation + store); sprinkled between the attention
    # pipeline stages of later batches to fill PE/Scalar gaps
    pending_p3 = []

    def make_p3_unit(ts, cc):
        def emit(sel=0):
            w = ts.stop - ts.start
            if sel == 1:
                # borrow an attention-output psum slot: legal at drain
                # positions that precede this batch's AV groups, and at the
                # tail — spreads the unit psum rotation across two tags
                t = ps_po.tile([128, 2, 256], F32, tag="pspo", name=f"ps3p_{ts.start}_{cc}")
                ps = t.rearrange("p a b -> p (a b)")
            else:
                ps = ps_mm.tile([128, TCS], F32, tag="ps1", name=f"ps3_{ts.start}_{cc}")
            for k in range(KC):
                nc.tensor.matmul(
                    ps[:, 0:w],
                    wp[k][:, cc * 128 : (cc + 1) * 128],
                    aoT[:, k, ts],
                    start=(k == 0),
                    stop=(k == KC - 1),
                )
            stg = ostg_pool.tile([128, TCS], F32, tag="stg", name=f"stg{ts.start}_{cc}")
            # psum evacuation with the proj bias fused (per-partition bias).
            # The narrow tail units (w < TCS) alternate onto DVE, which is
            # idle by then — consecutive units otherwise serialize on the
            # Scalar queue with no other PE work left to cover the wait.
            if w < TCS and cc % 2 == 1:
                nc.vector.tensor_scalar_add(
                    stg[:, 0:w], ps[:, 0:w], projb_cols[:, cc : cc + 1]
                )
            else:
                nc.scalar.activation(
                    stg[:, 0:w], ps[:, 0:w], ident,
                    bias=projb_cols[:, cc : cc + 1], scale=1.0,
                )
            nc.sync.dma_start(
                out=out_d[cc * 128 : (cc + 1) * 128, ts], in_=stg[:, 0:w]
            )

        return emit

    def drain_p3(n, sel=0):
        for _ in range(min(n, len(pending_p3))):
            pending_p3.pop(0)(sel)

    def att(b):
        et = et_pool.tile([128, 2, H, N], BF16, tag="et")
        ar = ar_pool.tile([128, H, N], F32, tag="ar")
        pss = {}

        def emit_scores(hg, b=b, et=et, pss=pss):
            for kt in range(2):
                nk = NK0 if kt == 0 else NK1
                kofs = b * N + kt * 128
                ps = ps_sc.tile([128, 4, 256], F32, tag="pssc")
                pss[(hg, kt)] = ps
                for j in range(4):
                    h = hg * 4 + j
                    off = (h % 2) * 64
                    # scoresT[nk, nq] = kT.T @ qT  (scale folded into Wq);
                    # psum slice SIG4[j] so concurrent row-packed MMs use
                    # different banks
                    nc.tensor.matmul(
                        ps[:nk, SIG4[j], 0:N],
                        qkT[6 + h // 2][off : off + 64, kofs : kofs + nk],
                        qkT[h // 2][off : off + 64, b * N : b * N + N],
                        start=True,
                        stop=True,
                    )
                raw = raw_pool.tile([128, 4, N], BF16, tag="raw")
                nc.scalar.activation(raw[:nk, :, :], ps[:nk, :, 0:N], exp)
                # multiplicative relative-position bias: et = exp(z)*exp(b).
                # kt0 runs on GpSimd (slow but fully covered by the kt1
                # chain), kt1 on DVE (den accumulates kt1 first).
                eng = nc.gpsimd if kt == 0 else nc.vector
                eng.tensor_tensor(
                    et[:nk, kt, hg * 4 : (hg + 1) * 4, :],
                    raw[:nk, :, :],
                    bias_sb[kt][:nk, hg * 4 : (hg + 1) * 4, :],
                    mybir.AluOpType.mult,
                )

        def emit_den(hg, et=et, ar=ar, pss=pss):
            # softmax denominators: ones-row matmuls (reduce + broadcast
            # across partitions in one PE op), accumulated over both nk
            # tiles into the kt-1 scores psum AFTER its exp consumed it.
            # kt1 first: its DVE multiply finishes well before kt0's
            # GpSimd multiply.
            den_flat = pss[(hg, 1)].rearrange("p a b -> p (a b)")
            for kt in (1, 0):
                nk = NK0 if kt == 0 else NK1
                for pr in range(2):
                    nc.tensor.matmul(
                        den_flat[:, pr * 512 : pr * 512 + 2 * N],
                        ones128[:nk, :],
                        et[:nk, kt, hg * 4 + 2 * pr : hg * 4 + 2 * pr + 2, :],
                        start=(kt == 1),
                        stop=(kt == 0),
                    )
            # reciprocal of the denominators (identical on every partition),
            # one op per head group (pr regions via 512-stride 3D AP)
            nc.vector.reciprocal_approx_fast(
                out=ar[:, hg * 4 : hg * 4 + 4, :],
                in_=bass.AP(
                    tensor=den_flat.tensor,
                    offset=den_flat.offset,
                    ap=[den_flat.ap[0], [512, 2], [1, 2 * N]],
                ),
            )

        def emit_av(grp, b=b, et=et, ar=ar):
            po = ps_po.tile([128, 2, 256], F32, tag="pspo")
            for j in range(4):
                h = grp * 4 + j
                base = (j % 2) * 64
                sl = j // 2
                for kt in range(2):
                    nk = NK0 if kt == 0 else NK1
                    nc.tensor.matmul(
                        po[base : base + 64, sl, 0:N],
                        v_sb[b][kt][:nk, h * 64 : (h + 1) * 64],
                        et[:nk, kt, sig(h), :],
                        start=(kt == 0),
                        stop=(kt == 1),
                        tile_position=(0, base),
                    )
            # normalize + evacuate, merged per row half: heads in row half
            # 0:64 of this po tile are (4g, 4g+2) whose 1/den live in the
            # ADJACENT ar slots (4g, 4g+1); row half 64:128 holds heads
            # (4g+1, 4g+3) = ar slots (4g+2, 4g+3).  aoT chunk index is
            # 2g + sl for both halves.
            for half in range(2):
                r0 = half * 64
                nc.vector.tensor_tensor(
                    aoT[r0 : r0 + 64, 2 * grp : 2 * grp + 2, b * N : b * N + N],
                    po[r0 : r0 + 64, 0:2, 0:N],
                    ar[r0 : r0 + 64, 4 * grp + 2 * half : 4 * grp + 2 * half + 2, :],
                    mybir.AluOpType.mult,
                )

        # software-pipelined stage order: den lags its head group by one
        # stage so the exp/multiply chain is covered by other PE work;
        # deferred phase-3 units fill the remaining gaps.
        # the last two batches drain extra phase-3 units so only batch 7's
        # own output chunk is left for the tail
        dn = 2 if b >= 6 else 1
        emit_scores(0)
        emit_scores(1)
        drain_p3(1, 1)
        emit_den(0)
        emit_scores(2)
        drain_p3(1, 1)
        emit_den(1)
        emit_av(0)
        drain_p3(dn)
        emit_den(2)
        emit_av(1)
        drain_p3(dn)
        emit_av(2)
        # ---- phase 3: token chunk t covers batches 2t,2t+1; the last
        # chunk is split per batch so batch 6's output drains during
        # att(b7) and only batch 7's is left for the tail ----
        if b % 2 == 1 and b < 6:
            for cc in range(KC):
                pending_p3.append(make_p3_unit(slice((b // 2) * TCS, (b // 2 + 1) * TCS), cc))
        elif b >= 6:
            for cc in range(KC):
                pending_p3.append(make_p3_unit(slice(b * N, (b + 1) * N), cc))

    # ---- main loop: interleave phase-1 blocks with the attention batches
    # they feed, so the scalar/vector/gpsimd attention work hides under the
    # PE-saturated projection matmuls ----
    for mt in range(MT):
        emit_p1_mt(mt)
        att(2 * mt)
        att(2 * mt + 1)
    n_tail = len(pending_p3)
    for i in range(n_tail):
        drain_p3(1, i % 2)

    for pool in (
        ostg_pool,
        ps_po,
        ps_sc,
        ar_pool,
        raw_pool,
        et_pool,
        ao_pool,
        wp_pool,
        wq_pool,
        xt_pool,
        ps_mm,
        v_pool,
        qk_pool,
        consts,
    ):
        pool.release()


def _get_compiled():
    if "nc" not in _COMPILED:
        _COMPILED["nc"] = _build_nc()
    return _COMPILED["nc"]


def _prep_host(inputs):
    qkv_w = np.asarray(inputs["qkv_w"], dtype=np.float32)
    proj_w = np.asarray(inputs["proj_w"], dtype=np.float32)
    proj_b = np.asarray(inputs["proj_b"], dtype=np.float32)
    rel_table = np.asarray(inputs["rel_table"], dtype=np.float32)
    rel_index = np.asarray(inputs["rel_index"]).astype(np.int64)
    # match jax gather semantics (clamps out-of-range indices)
    rel_index = np.clip(rel_index, 0, rel_table.shape[0] - 1)

    w = qkv_w.copy()
    w[:C] *= SCALE  # fold the attention scale into Wq
    wqkvT = np.ascontiguousarray(w.T).astype(ml_dtypes.bfloat16)
    wprojT = np.ascontiguousarray(proj_w.T).astype(ml_dtypes.bfloat16)
    projb2 = np.ascontiguousarray(proj_b.reshape(1, C))

    bias_full = rel_table[rel_index]          # [nq, nk, H]
    biasT = np.exp(bias_full.transpose(2, 1, 0))  # [H, nk, nq], exp for the
    # multiplicative-bias trick: exp(z + b) = exp(z) * exp(b)
    perm = [(t // 4) * 4 + (0, 2, 1, 3)[t % 4] for t in range(H)]
    biasT = biasT[perm]
    pad = np.zeros((H, 2 * 128, N), np.float32)
    pad[:, :N, :] = biasT
    bias_dev = np.ascontiguousarray(
        pad.reshape(H, 2, 128, N).transpose(1, 2, 0, 3)
    ).astype(ml_dtypes.bfloat16)
    return wqkvT, wprojT, projb2, bias_dev


def _in_maps(inputs):
    x = np.asarray(inputs["x"], dtype=np.float32)
    wqkvT, wprojT, projb2, bias_dev = _prep_host(inputs)
    maps = []
    for i in range(N_CORES):
        shard = x[i * BL : (i + 1) * BL].reshape(M, C)
        maps.append(
            {
                "xT": np.ascontiguousarray(shard.T).astype(ml_dtypes.bfloat16),
                "wqkvT": wqkvT,
                "wprojT": wprojT,
                "projb": projb2,
                "biasT": bias_dev,
            }
        )
    return maps


def _run(inputs, trace=False):
    nc = _get_compiled()
    maps = _in_maps(inputs)
    last_err = None
    for attempt in range(3):
        try:
            res = run_bass_kernel_spmd(
                nc, maps, core_ids=list(range(N_CORES)), trace=trace
            )
            break
        except Exception as e:  # transient device/runtime hiccups
            last_err = e
            time.sleep(15 * (attempt + 1))
    else:
        raise last_err
    out = np.empty((B, N, C), dtype=np.float32)
    for i in range(N_CORES):
        # device output is yT [C, M]; transpose back on the host
        out[i * BL : (i + 1) * BL] = (
            res.results[i]["out"].reshape(C, BL, N).transpose(1, 2, 0)
        )
    return out, res


def kernel(**inputs):
    out, _ = _run(inputs, trace=False)
    return out


def run_traced(**inputs):
    """Like kernel() but with NTFF tracing; returns (out, BassKernelResults)."""
    return _run(inputs, trace=True)
